# revision 8
# baseline (speedup 1.0000x reference)
"""Trainium2 Bass kernel for nn_CausalSelfAttention_16003048145608 (v2).

Reference semantics: B=4, T=1024, C=2048, H=16 heads, HD=128.
  qkv = x @ W_attn ; split q,k,v ; input-dependent RoPE positions t derived
  from a histogram of token_index over ALL batch rows + per-row gather/cumsum;
  RoPE(q,k) with per-token angle t; q[...,-1]=1, k[...,-1]=cumulative_scores;
  v *= exp(cumulative_scores); causal softmax(q k^T/sqrt(HD)) @ v; y @ W_proj.

Sharding (8 cores): batch(4) x head-group(2 groups of 8 heads); per-core
partial proj outputs summed pairwise on the host (only cross-core reduction).

v2 design:
  * RoPE positions + trig tables are computed on the HOST in f64 (cheap numpy)
    and shipped as cos/sin tables -- removes a long serial on-device chain.
  * The QKV projections run as fp8e4(E4M3) DoubleRow matmuls (4x fp16 MAC
    rate, verified on HW) with a 3-term residual split for near-fp16 accuracy:
        x @ W ~= x_hi@W_hi + x_lo@W_hi + x_hi@W_lo
    where x_hi = fp8(x), x_lo = fp8(x - x_hi), W scaled by 64 (host-side) to
    keep fp8 normal range; the 64x is descaled via the softmax scale (q,k)
    and the exp(cs)/64 v-scale.  Net cost: 0.75x of the fp16 matmul.
  * Attention in fp16: transposed scores sT(k,q) = kT.T @ q per 512-wide q
    block, exp on ACT (range-trimmed to the causal region), diagonal-only
    affine_select masking on Pool, softmax denominator via ones-matmul,
    PV accumulation in PSUM, 1/z fused into the PSUM->SBUF move.
  * Output projection in fp16 (y quantization to fp8 would cost more DVE than
    it saves in PE).
  * Host pre-lays-out every operand so each SBUF tensor loads with one DMA.
"""

import math
import numpy as np
import ml_dtypes

import concourse.bass as bass
import concourse.bacc as bacc
import concourse.tile as tile
from concourse import mybir
from concourse.bass_utils import run_bass_kernel_spmd

F32 = mybir.dt.float32
FP16 = mybir.dt.float16
FP8 = mybir.dt.float8e4
ALU = mybir.AluOpType
ACTF = mybir.ActivationFunctionType
PM_DR = mybir.MatmulPerfMode.DoubleRow

B, T, C, H, HD = 4, 1024, 2048, 16, 128
NHC = 8                  # heads per core
NCORES = 8
KP = C // 256            # 8 fp8-DoubleRow contraction pairs (K=256 each)
QTILES = T // 128        # 8
WS = 64.0                # host-side weight scale (fp8 normal range)
SCALE_ACT = 1.0 / (math.sqrt(HD) * WS * WS)   # folds 1/(64*64) descale

NP_FP8 = ml_dtypes.float8_e4m3
NP_FP16 = np.float16

REPEAT = 1   # emit the body N times (for slope-based HW timing)


def _emit(nc):
    # ---- DRAM I/O (per-core shapes; host feeds per-core slices) ----
    x_hi = nc.dram_tensor("x_hi", [128, KP * 2 * T], FP8, kind="ExternalInput").ap()
    x_lo = nc.dram_tensor("x_lo", [128, KP * 2 * T], FP8, kind="ExternalInput").ap()
    wqk_hi = nc.dram_tensor("wqk_hi", [128, NHC * KP * 2 * 256], FP8,
                            kind="ExternalInput").ap()
    wqk_lo = nc.dram_tensor("wqk_lo", [128, NHC * KP * 2 * 256], FP8,
                            kind="ExternalInput").ap()
    wv_hi = nc.dram_tensor("wv_hi", [128, KP * 2 * NHC * HD], FP8,
                           kind="ExternalInput").ap()
    wv_lo = nc.dram_tensor("wv_lo", [128, KP * 2 * NHC * HD], FP8,
                           kind="ExternalInput").ap()
    w_p = nc.dram_tensor("w_p", [128, NHC * C], FP16, kind="ExternalInput").ap()
    cos_t = nc.dram_tensor("cos_t", [128, T], FP16, kind="ExternalInput").ap()
    sin_t = nc.dram_tensor("sin_t", [128, T], FP16, kind="ExternalInput").ap()
    klr = nc.dram_tensor("klr", [1, T], FP16, kind="ExternalInput").ap()
    evs = nc.dram_tensor("evs", [128, QTILES], F32, kind="ExternalInput").ap()
    out = nc.dram_tensor("out", [T, C], FP16, kind="ExternalOutput").ap()

    with tile.TileContext(nc) as tc:
        for _ in range(REPEAT):
            _body(nc, tc, x_hi, x_lo, wqk_hi, wqk_lo, wv_hi, wv_lo, w_p,
                  cos_t, sin_t, klr, evs, out)
    return nc


def _body(nc, tc, x_hi, x_lo, wqk_hi, wqk_lo, wv_hi, wv_lo, w_p,
          cos_t, sin_t, klr, evs, out):
    from contextlib import ExitStack

    with ExitStack() as ctx:
        persist = ctx.enter_context(tc.tile_pool(name="persist", bufs=1))
        ps = ctx.enter_context(tc.tile_pool(name="ps", bufs=4, space="PSUM"))
        pt_pool = ctx.enter_context(tc.tile_pool(name="pt", bufs=14))

        # ---------- persistent tiles ----------
        q_all = persist.tile([128, NHC * T], FP16, tag="q_all")
        k_all = persist.tile([128, NHC * T], FP16, tag="k_all")
        v_all = persist.tile([128, QTILES * NHC * HD], FP16, tag="v_all")
        y_big = persist.tile([128, NHC * T], FP16, tag="y_big")
        cos_sb = persist.tile([128, T], FP16, tag="cos_sb")
        sin_sb = persist.tile([128, T], FP16, tag="sin_sb")
        klr_sb = persist.tile([1, T], FP16, tag="klr_sb")
        evs_sb = persist.tile([128, QTILES], F32, tag="evs_sb")
        ones_sq = persist.tile([128, 128], FP16, tag="ones_sq")
        one_row = persist.tile([1, T], FP16, tag="one_row")
        nc.vector.memset(ones_sq[:], 1.0)
        nc.vector.memset(one_row[:], WS)   # q last-channel override (64*1)

        nc.sync.dma_start(cos_sb[:], cos_t)
        nc.sync.dma_start(sin_sb[:], sin_t)
        nc.sync.dma_start(klr_sb[:], klr)
        nc.sync.dma_start(evs_sb[:], evs)

        with tc.tile_pool(name="bigin", bufs=1) as bigin, \
             tc.tile_pool(name="wstream", bufs=2) as wst, \
             tc.tile_pool(name="rope", bufs=2) as rope_pool, \
             tc.tile_pool(name="attp", bufs=4) as attp:
            xh_sb = bigin.tile([128, KP * 2 * T], FP8, tag="xh_sb")
            xl_sb = bigin.tile([128, KP * 2 * T], FP8, tag="xl_sb")
            wvh_sb = bigin.tile([128, KP * 2 * NHC * HD], FP8, tag="wvh_sb")
            wvl_sb = bigin.tile([128, KP * 2 * NHC * HD], FP8, tag="wvl_sb")
            # DMA queue order = first-need order for the qk0 term sequence
            # (hi_w*hi_x, lo_w*hi_x, hi_w*lo_x): wh0, x_hi, wl0, wv_hi, x_lo.
            wh0 = wst.tile([128, KP * 2 * 256], FP8, tag="wh", name="wh0")
            wl0 = wst.tile([128, KP * 2 * 256], FP8, tag="wl", name="wl0")
            nc.sync.dma_start(wh0[:], wqk_hi[:, 0:KP * 512])
            HX = KP * T          # half of the (kp,j,t) free extent
            nc.sync.dma_start(xh_sb[:, 0:HX], x_hi[:, 0:HX])
            nc.sync.dma_start(xh_sb[:, HX:2 * HX], x_hi[:, HX:2 * HX])
            nc.sync.dma_start(wl0[:], wqk_lo[:, 0:KP * 512])
            nc.sync.dma_start(wvh_sb[:], wv_hi)
            nc.sync.dma_start(xl_sb[:, 0:HX], x_lo[:, 0:HX])
            nc.sync.dma_start(xl_sb[:, HX:2 * HX], x_lo[:, HX:2 * HX])
            nc.sync.dma_start(wvl_sb[:], wv_lo)

            xh = xh_sb[:].rearrange("p (kp j t) -> p kp j t", kp=KP, j=2)
            xl = xl_sb[:].rearrange("p (kp j t) -> p kp j t", kp=KP, j=2)
            wvh = wvh_sb[:].rearrange("p (kp j n) -> p kp j n", kp=KP, j=2)
            wvl = wvl_sb[:].rearrange("p (kp j n) -> p kp j n", kp=KP, j=2)

            def emit_v_half(nc2):
                # v token-major: out(tok, vcol) = x_tile.T @ w_v
                # PSUM tag "att" (not "mm"): the 4 "mm" bufs hold a full head's
                # qk tiles until RoPE reads them; rotating v tiles through
                # "mm" would deadlock PE (WAR) against later DVE queue entries.
                for mt in range(QTILES):
                    vv_ps = ps.tile([128, 512], F32, tag="att", name="vv_ps")
                    terms = [
                        (lambda kp, m=mt: xh[:, kp, :, m * 128:(m + 1) * 128],
                         lambda kp, n=nc2: wvh[:, kp, :, n * 512:(n + 1) * 512]),
                        (lambda kp, m=mt: xl[:, kp, :, m * 128:(m + 1) * 128],
                         lambda kp, n=nc2: wvh[:, kp, :, n * 512:(n + 1) * 512]),
                        (lambda kp, m=mt: xh[:, kp, :, m * 128:(m + 1) * 128],
                         lambda kp, n=nc2: wvl[:, kp, :, n * 512:(n + 1) * 512]),
                    ]
                    for ti, (lf, rf) in enumerate(terms):
                        for kp in range(KP):
                            nc.tensor.matmul(
                                vv_ps[:], lf(kp), rf(kp),
                                start=(ti == 0 and kp == 0),
                                stop=(ti == 2 and kp == KP - 1),
                                perf_mode=PM_DR)
                    # scale by exp(cs)/64 (per-token = per-partition column)
                    nc.vector.tensor_scalar(
                        v_all[:, mt * 1024 + nc2 * 512: mt * 1024 + nc2 * 512 + 512],
                        vv_ps[:], evs_sb[:, mt:mt + 1], None, ALU.mult)

            def emit_qk(h, pre=None):
                if pre is not None:
                    wh, wl = pre
                else:
                    wh = wst.tile([128, KP * 2 * 256], FP8, tag="wh", name="wh")
                    wl = wst.tile([128, KP * 2 * 256], FP8, tag="wl", name="wl")
                    nc.sync.dma_start(wh[:],
                                      wqk_hi[:, h * KP * 512:(h + 1) * KP * 512])
                    nc.sync.dma_start(wl[:],
                                      wqk_lo[:, h * KP * 512:(h + 1) * KP * 512])
                whv = wh[:].rearrange("p (kp j n) -> p kp j n", kp=KP, j=2)
                wlv = wl[:].rearrange("p (kp j n) -> p kp j n", kp=KP, j=2)
                qk_ps = [ps.tile([128, 512], F32, tag="mm", name="qk_ps")
                         for _ in range(4)]  # q0 q1 k0 k1
                for t2 in range(2):          # 0 = q cols, 1 = k cols
                    for nc2 in range(2):
                        dst = qk_ps[t2 * 2 + nc2][:]
                        terms = [
                            (lambda kp, t=t2: whv[:, kp, :, t * 128:(t + 1) * 128],
                             lambda kp, n=nc2: xh[:, kp, :, n * 512:(n + 1) * 512]),
                            (lambda kp, t=t2: wlv[:, kp, :, t * 128:(t + 1) * 128],
                             lambda kp, n=nc2: xh[:, kp, :, n * 512:(n + 1) * 512]),
                            (lambda kp, t=t2: whv[:, kp, :, t * 128:(t + 1) * 128],
                             lambda kp, n=nc2: xl[:, kp, :, n * 512:(n + 1) * 512]),
                        ]
                        for ti, (lf, rf) in enumerate(terms):
                            for kp in range(KP):
                                nc.tensor.matmul(
                                    dst, lf(kp), rf(kp),
                                    start=(ti == 0 and kp == 0),
                                    stop=(ti == 2 and kp == KP - 1),
                                    perf_mode=PM_DR)
                return qk_ps

            def emit_rope(h, qk_ps):
                # per 512-column half so scores qc=0 (which reads only cols
                # 0:512 of q_all/k_all) starts before the second half lands
                for nc2 in range(2):
                    hs = slice(nc2 * 512, (nc2 + 1) * 512)
                    gs = slice(h * T + nc2 * 512, h * T + (nc2 + 1) * 512)
                    for t2, dst in ((0, q_all), (1, k_all)):
                        raw = rope_pool.tile([128, 512], FP16, tag="raw",
                                             name="raw")
                        # PSUM->SBUF fp16 move on ACT: frees DVE for the
                        # cos/sin multiplies so rope keeps up with PE
                        nc.scalar.activation(raw[:], qk_ps[t2 * 2 + nc2][:],
                                             ACTF.Copy)
                        rot = rope_pool.tile([128, 512], FP16, tag="rot",
                                             name="rot")
                        nc.sync.dma_start(rot[0:64, :], raw[64:128, :])
                        nc.sync.dma_start(rot[64:128, :], raw[0:64, :])
                        tmp = rope_pool.tile([128, 512], FP16, tag="tmp",
                                             name="tmp")
                        nc.vector.tensor_tensor(tmp[:], raw[:], cos_sb[:, hs],
                                                ALU.mult)
                        nc.vector.tensor_tensor(rot[:], rot[:], sin_sb[:, hs],
                                                ALU.mult)
                        nc.vector.tensor_tensor(dst[:, gs], tmp[:], rot[:],
                                                ALU.add)
                    # last-rotary-channel overrides (partition 127 via DMA)
                    nc.sync.dma_start(q_all[127:128, gs], one_row[:, hs])
                    nc.sync.dma_start(k_all[127:128, gs], klr_sb[:, hs])

            att_state = {}

            def emit_scores(h):
                """Scores matmuls + exp + causal select + denominator sums.
                z/PV are deferred to emit_zpv so the ACT/DVE latency hides
                behind the next head's qk block instead of stalling PE."""
                sl_h = h * T
                st = {}
                for qc in range(2):
                    ktmax = (qc + 1) * 4
                    qh = q_all[:, sl_h + qc * 512: sl_h + qc * 512 + 512]
                    p_tiles = []
                    for kt in range(ktmax):
                        s_ps = ps.tile([128, 512], F32, tag="att", name="s_ps")
                        nc.tensor.matmul(
                            s_ps[:],
                            k_all[:, sl_h + kt * 128: sl_h + kt * 128 + 128],
                            qh, start=True, stop=True)
                        p_sb = pt_pool.tile([128, 512], FP16, tag="p", name="p_sb")
                        lo = kt * 128 - qc * 512
                        if lo >= 0:   # causal-crossing tile
                            if lo > 0:
                                nc.vector.memset(p_sb[:, 0:lo], 0.0)
                            nc.scalar.activation(p_sb[:, lo:512], s_ps[:, lo:512],
                                                 ACTF.Exp, scale=SCALE_ACT)
                            nc.gpsimd.affine_select(
                                p_sb[:, lo:lo + 128], p_sb[:, lo:lo + 128],
                                [[1, 128]], ALU.is_ge, 0.0,
                                base=0, channel_multiplier=-1)
                        else:
                            nc.scalar.activation(p_sb[:], s_ps[:], ACTF.Exp,
                                                 scale=SCALE_ACT)
                        p_tiles.append(p_sb)

                    s_acc = attp.tile([128, 512], FP16, tag="s_acc")
                    nc.vector.tensor_tensor(s_acc[:], p_tiles[0][:],
                                            p_tiles[1][:], ALU.add)
                    for kt in range(2, ktmax):
                        nc.vector.tensor_tensor(s_acc[:], s_acc[:],
                                                p_tiles[kt][:], ALU.add)
                    st[qc] = (p_tiles, s_acc)
                att_state[h] = st

            def emit_zpv(h):
                sl_h = h * T
                for qc in range(2):
                    p_tiles, s_acc = att_state[h][qc]
                    ktmax = (qc + 1) * 4
                    z_ps = ps.tile([128, 512], F32, tag="att", name="z_ps")
                    nc.tensor.matmul(z_ps[:], ones_sq[:], s_acc[:],
                                     start=True, stop=True)
                    rz32 = attp.tile([128, 512], F32, tag="rz32")
                    nc.vector.reciprocal_approx_fast(rz32[:], z_ps[:])
                    y_ps = ps.tile([128, 512], F32, tag="att", name="y_ps")
                    for kt in range(ktmax):
                        nc.tensor.matmul(
                            y_ps[:],
                            v_all[:, kt * 1024 + h * 128: kt * 1024 + h * 128 + 128],
                            p_tiles[kt][:], start=(kt == 0), stop=(kt == ktmax - 1))
                    nc.vector.tensor_tensor(
                        y_big[:, sl_h + qc * 512: sl_h + qc * 512 + 512],
                        y_ps[:], rz32[:], ALU.mult)
                del att_state[h]

            # ---------- emission: z/PV deferred one qk-block back ----------
            qk_pending = emit_qk(0, pre=(wh0, wl0))
            emit_v_half(0)
            emit_rope(0, qk_pending)
            wp_half = [None, None]
            for h in range(NHC):
                emit_scores(h)
                if h + 1 < NHC:
                    qk_pending = emit_qk(h + 1)
                if h == 0:
                    emit_v_half(1)
                if h + 1 < NHC:
                    emit_rope(h + 1, qk_pending)
                emit_zpv(h)
                if h == NHC - 2:   # prefetch first w_p half during head 7
                    wp_half[0] = persist.tile([128, 4 * C], FP16, tag="wp0",
                                              name="wp0")
                    nc.sync.dma_start(wp_half[0][:], w_p[:, 0:4 * C])

        # ================= output projection (partial) =================
        with tc.tile_pool(name="wpp", bufs=1) as wpp, \
             tc.tile_pool(name="outp", bufs=3) as outp:
            wp_half[1] = wpp.tile([128, 4 * C], FP16, tag="wp1", name="wp1")
            for qtr in range(4):   # quarter DMAs: h8=4 stalls ~3us less
                nc.sync.dma_start(wp_half[1][:, qtr * C:(qtr + 1) * C],
                                  w_p[:, (4 + qtr) * C:(5 + qtr) * C])
            for qt in range(QTILES):
                for n4 in range(4):
                    o_ps = ps.tile([128, 512], F32, tag="mm", name="o_ps")
                    for h8 in range(NHC):
                        wsrc = wp_half[h8 // 4]
                        nc.tensor.matmul(
                            o_ps[:],
                            y_big[:, h8 * T + qt * 128: h8 * T + qt * 128 + 128],
                            wsrc[:, (h8 % 4) * C + n4 * 512:
                                 (h8 % 4) * C + n4 * 512 + 512],
                            start=(h8 == 0), stop=(h8 == NHC - 1))
                    o_sb = outp.tile([128, 512], FP16, tag="o_sb")
                    nc.scalar.activation(o_sb[:], o_ps[:], ACTF.Copy)
                    nc.sync.dma_start(
                        out[qt * 128:(qt + 1) * 128, n4 * 512:(n4 + 1) * 512],
                        o_sb[:])


_NC_CACHE = None


def _get_nc():
    global _NC_CACHE
    if _NC_CACHE is None:
        nc = bacc.Bacc("TRN2", target_bir_lowering=False, debug=False,
                       num_devices=NCORES)
        _emit(nc)
        nc.compile()
        _NC_CACHE = nc
    return _NC_CACHE


def _fp8_split(a):
    """Return (hi, lo) fp8e4m3 arrays with hi + lo ~= a."""
    hi = a.astype(NP_FP8)
    lo = (a - hi.astype(np.float32)).astype(NP_FP8)
    return hi, lo


def _dr_layout(mT):
    """[C, N] -> [128, KP*2*N]: element (p, kp, j, n) = mT[kp*256+j*128+p, n]."""
    Cdim, N = mT.shape
    return np.ascontiguousarray(
        mT.reshape(KP, 2, 128, N).transpose(2, 0, 1, 3).reshape(128, KP * 2 * N))


def make_in_maps(x, cumulative_scores, token_index, padding_mask, W_attn, W_proj):
    x = np.asarray(x, np.float32)
    csf = np.asarray(cumulative_scores, np.float32)
    tok = np.asarray(token_index, np.int64)
    padf = np.asarray(padding_mask, np.float32)
    Wa = np.asarray(W_attn, np.float32)
    Wp = np.asarray(W_proj, np.float32)

    # ---------------- host positions + trig (f64) ----------------
    counts = np.zeros((1024,), np.float64)
    np.add.at(counts, tok.reshape(-1), 1.0)
    recip = 1.0 / (counts + 1e-10)
    t = np.cumsum(recip[tok], axis=-1)            # (B,T)
    inv_freq = 1.0 / (10000.0 ** (np.arange(0, HD, 2, dtype=np.float64) / HD))
    ang = t[:, None, :] * np.concatenate([inv_freq, inv_freq])[None, :, None]
    cos_tab = np.cos(ang).astype(NP_FP16)         # (B,128,T)
    sin_tab = np.sin(ang).astype(np.float32)
    sin_tab[:, 0:64, :] *= -1.0                   # fold rotate_half sign
    sin_tab = sin_tab.astype(NP_FP16)

    klr_all = np.where(padf > 0, WS * csf, -60000.0).astype(NP_FP16)  # (B,T)
    evs_all = (np.exp(csf) / WS).astype(np.float32)                   # (B,T)

    # ---------------- fp8 operand splits ----------------
    xT = np.ascontiguousarray(x.transpose(0, 2, 1))                   # (B,C,T)
    Wq, Wk, Wv = Wa[:, 0:C], Wa[:, C:2 * C], Wa[:, 2 * C:3 * C]

    in_maps = []
    for core in range(NCORES):
        b, hg = core // 2, core % 2
        cols = slice(hg * 1024, (hg + 1) * 1024)
        xh, xl = _fp8_split(xT[b])
        # w_qk per head: [C, 256] = [q_h | k_h], scaled by WS
        wqk = np.concatenate(
            [np.concatenate([Wq[:, cols][:, hh * 128:(hh + 1) * 128],
                             Wk[:, cols][:, hh * 128:(hh + 1) * 128]], axis=1)
             for hh in range(NHC)], axis=1) * WS                      # (C, 8*256)
        wqk_hi, wqk_lo = _fp8_split(wqk)
        wv = Wv[:, cols] * WS                                         # (C, 1024)
        wv_hi, wv_lo = _fp8_split(wv)

        # DR layouts; w_qk laid out head-major for per-head streaming
        wqk_hi_l = np.concatenate(
            [_dr_layout(wqk_hi.astype(np.float32)[:, hh * 256:(hh + 1) * 256]
                        .astype(NP_FP8)) for hh in range(NHC)], axis=1)
        wqk_lo_l = np.concatenate(
            [_dr_layout(wqk_lo.astype(np.float32)[:, hh * 256:(hh + 1) * 256]
                        .astype(NP_FP8)) for hh in range(NHC)], axis=1)

        in_maps.append({
            "x_hi": _dr_layout(xh),
            "x_lo": _dr_layout(xl),
            "wqk_hi": wqk_hi_l,
            "wqk_lo": wqk_lo_l,
            "wv_hi": _dr_layout(wv_hi),
            "wv_lo": _dr_layout(wv_lo),
            "w_p": np.ascontiguousarray(
                Wp[hg * 1024:(hg + 1) * 1024, :].reshape(NHC, 128, C)
                .transpose(1, 0, 2).reshape(128, NHC * C)).astype(NP_FP16),
            "cos_t": np.ascontiguousarray(cos_tab[b]),
            "sin_t": np.ascontiguousarray(sin_tab[b]),
            "klr": np.ascontiguousarray(klr_all[b][None, :]),
            "evs": np.ascontiguousarray(
                evs_all[b].reshape(QTILES, 128).T),
        })
    return in_maps


def kernel(x, cumulative_scores, token_index, padding_mask, W_attn, W_proj):
    nc = _get_nc()
    in_maps = make_in_maps(x, cumulative_scores, token_index, padding_mask,
                           W_attn, W_proj)
    res = run_bass_kernel_spmd(nc, in_maps, list(range(NCORES)))
    outs = [res.results[c]["out"].astype(np.float32) for c in range(NCORES)]
    full = np.stack([outs[2 * b] + outs[2 * b + 1] for b in range(B)], axis=0)
    return full.astype(np.float32)
